# revision 24
# baseline (speedup 1.0000x reference)
"""Trainium2 Bass kernel for a supervised-contrastive-style loss.

Reference computation (see problem statement):
  - dropout(p=0.5, scale 2, jax key 42) on gathered class-2 rows, concat -> feats [N2, D]
  - fn = feats / max(||feats||, 1e-8);  sim = fn @ fn.T / T
  - denom_i = sum_j exp(sim_ij) * [labs_i == labs_j]
  - loss = -mean(sim_ii - log denom_i)

Strategy (v2):
  * Host: mirror the reference prologue (dropout/concat/normalize) op-for-op on
    the default jax backend (bit-identical PRNG + fn), then sort rows by class.
    The label mask becomes block-diagonal, so the device only computes
    same-class row x col tiles (~46% of the full N2^2 work).
  * fn is scaled by 64 and quantized to fp8-e4m3 (validated: loss rel err
    ~1e-3 << 2e-2 tolerance).  The ENTIRE fn matrix lives in SBUF (fp8 is
    ~88KB/partition), so column panels are DMA'd from HBM exactly once per
    iteration instead of once per row-tile -- the f32 baseline was HBM-bound
    (42MB/core/rep at ~358GB/s ~ 118us).
  * Main matmuls run fp8 + DoubleRow (contraction 256/instruction, 2x PE
    throughput vs bf16/f32r).
  * Symmetry: within a class block sim is symmetric, so only upper-triangle
    (row-tile[128] x col-panel[<=512]) tiles are computed.  Each tile yields a
    row-sum (ScalarE exp activation with accum_out) and, for strictly-upper
    tiles, a column-sum (ones-vector matmul of the exp tile).  Rows are dealt
    to the 8 cores with a stride-8 "comb" (core k owns tiles k, k+8, ...) so
    every core runs the *same* staircase program; the few below-diagonal jobs
    this over-approximates are simply ignored on the host.
  * A small diag pass recomputes each owned row-tile's diagonal block with
    identical operands and extracts raw sim_ii (bit-identical to the value
    that went through exp), so log(denom_i) - sim_ii cancels structurally.
  * Host: float64 combination of row/col partials, dead-row corrections, log,
    mean.
"""

import math

import numpy as np
import ml_dtypes

TEMPERATURE = 0.07
DROP_P = 0.5
EPS = 1e-8
NCORES = 8
KP = 128     # partition size
PANEL = 512  # max matmul moving free dim (one PSUM bank of fp32)
FSCALE = 64.0  # fn pre-scale before fp8 quantization (power of 2)
F8 = ml_dtypes.float8_e4m3

_CACHE = {}


# --------------------------------------------------------------------------
# host-side preparation
# --------------------------------------------------------------------------

def _host_prep(features, labels, aug_indices):
    """Mirror the reference's prologue op-for-op on the default jax backend so
    the dropout PRNG bits and fn values match the graded reference exactly."""
    import jax
    import jax.numpy as jnp

    features = jnp.asarray(np.asarray(features))
    labels_np = np.asarray(labels)
    aug_np = np.asarray(aug_indices)

    pert = features[jnp.asarray(aug_np)]
    keep = jax.random.bernoulli(jax.random.key(42), 1.0 - DROP_P, pert.shape)
    pert = jnp.where(keep, pert * 2.0, jnp.zeros((), dtype=pert.dtype))
    feats = jnp.concatenate([features, pert], axis=0)

    norms = jnp.sqrt(jnp.sum(feats * feats, axis=1, keepdims=True))
    fn = np.asarray(feats / jnp.maximum(norms, EPS)).astype(np.float32)
    labs = np.concatenate([labels_np, labels_np[aug_np]], axis=0)

    perm = np.argsort(labs, kind="stable")
    fn_sorted = np.ascontiguousarray(fn[perm])
    labs_sorted = labs[perm]
    return fn, labs, perm, fn_sorted, labs_sorted


class _Plan:
    """Compile-time structure shared by program builder, simulator, finisher.

    Per class c (counts in sorted-label order):
      RT[c]  global 128-row tiles;  R[c] = ceil(RT/8) per-core row slots
      P[c]   column panels;  w(c,p) widths (last panel exact)
    Core k's row slot (c, j) holds physical tile t = k + 8*j (dead if t>=RT).
    Structural jobs per class: {(p, j): j <= p//2, j < R[c]} — on core k the
    job is *counted* iff t real and p >= t//4 (upper-or-diagonal).
    """

    def __init__(self, n2, d, class_counts):
        assert d % KP == 0
        self.n2 = n2
        self.d = d
        self.kt = d // KP
        self.counts = list(class_counts)
        self.ncls = len(self.counts)
        self.RT = [math.ceil(c / KP) for c in self.counts]
        self.R = [math.ceil(rt / NCORES) for rt in self.RT]
        self.P = [math.ceil(c / PANEL) for c in self.counts]
        # last-panel widths, rounded up to even (the fp32r baseline required
        # an even moving free dim; DoubleRow keeps the same convention); the
        # extra zero column is corrected on host
        self.Wreal = [c - (p - 1) * PANEL for c, p in zip(self.counts, self.P)]
        self.W = [w + (w & 1) for w in self.Wreal]
        self.S = [r * KP for r in self.R]
        self.row_slots = sum(self.S)
        self.col_slots = sum(p * PANEL for p in self.P)
        self.nrt = sum(self.R)
        self.cls_row_off = np.cumsum([0] + self.counts).tolist()
        self.slot_off = np.cumsum([0] + self.S).tolist()
        self.panel_off = np.cumsum([0] + [p * PANEL for p in self.P]).tolist()

        # emission plan: per class, panels grouped G at a time descending
        # from the last panel.  Within a group, mm-set (j, glist) runs the
        # row-slot-j matmul against every panel in glist with the SAME
        # stationary weights (k2-major inner loop), so walrus's redundant-
        # load-weight optimization elides all but the first LDWEIGHTS of
        # each k2 sweep (LDW is otherwise ~equal to the matmul time for
        # fp8 DoubleRow and fully serializes with it).
        GRP = 3

        def njobs_of(c, p):
            return min(p // 2 + 1, self.R[c])

        self.njobs_of = njobs_of
        groups = []   # (c, [p desc], sets=[(j, glist)], work)
        for c in range(self.ncls):
            ps = list(range(self.P[c] - 1, -1, -1))
            for g0 in range(0, len(ps), GRP):
                gp = ps[g0:g0 + GRP]
                sets = []
                for j in range(njobs_of(c, gp[0])):
                    glist = [p for p in gp if njobs_of(c, p) > j]
                    sets.append((j, glist))
                work = sum(len(gl) for _, gl in sets)
                groups.append((c, gp, sets, work))
        groups.sort(key=lambda g: -g[3])
        self.group_seq = groups

        def width(c, p):
            return PANEL if p < self.P[c] - 1 else self.W[c]

        self.width = width
        self.jobs = []           # flat (c, p, j, w) in emission order
        for c, gp, sets, _ in groups:
            for j, glist in sets:
                for p in glist:
                    self.jobs.append((c, p, j, width(c, p)))
        self.njobs = len(self.jobs)
        self.job_id = {(c, p, j): i for i, (c, p, j, w) in
                       enumerate(self.jobs)}

        # per-set emission layout: slices in ASCENDING p order (so the only
        # narrow panel, the class's last, sits at the end -> the batched
        # activations read a gap-free span).  Slice 0 is the even-diagonal
        # panel p==2j when present; it gets its own activation/partial (cores
        # 4-7 exclude it from their denominators).  partial ids: pa = panel-2j
        # rowsum, pb = rest-of-set rowsum.  Colsums (strictly-upper panels
        # only) get sequential slots, packed 4 per PSUM bank for DMA.
        self.set_seq = []   # (c, j, slices=[(p, off, w)], pa, pb, span)
        self.emit_groups = []  # (c, col0, colw, [set_seq indices])
        self.ncsum = 0
        self.csum_slot = {}
        npart = 0
        for c, gp, sets, _ in groups:
            p_lo, p_hi = min(gp), max(gp)
            col0 = self.panel_off[c] + p_lo * PANEL
            colw = (p_hi - p_lo) * PANEL + width(c, p_hi)
            self.emit_groups.append(
                (c, col0, colw,
                 list(range(len(self.set_seq), len(self.set_seq) + len(sets)))))
            for j, glist in sets:
                slices = []
                off = 0
                for p in sorted(glist):
                    w = width(c, p)
                    slices.append((p, off, w))
                    off += PANEL
                span = slices[-1][1] + slices[-1][2]
                has_diag0 = slices[0][0] == 2 * j
                pa = pb = None
                if has_diag0:
                    pa = npart
                    npart += 1
                    if len(slices) > 1:
                        pb = npart
                        npart += 1
                else:
                    pb = npart
                    npart += 1
                for p, _o, _w in slices:
                    if p != 2 * j:
                        self.csum_slot[(c, p, j)] = self.ncsum
                        self.ncsum += 1
                self.set_seq.append((c, j, slices, pa, pb, span))
        self.npart = npart
        self.ncsum_pad = ((self.ncsum + 3) // 4) * 4
        self.setmax = max(len(s[2]) for s in self.set_seq)

    def rowtile_index(self, c, j):
        return sum(self.R[cc] for cc in range(c)) + j

    def phys_tile(self, core, j):
        return core + NCORES * j

    def realrows(self, c, t):
        return int(min(max(self.counts[c] - KP * t, 0), KP))


def _build_host_arrays(plan, fn_sorted):
    """fp8 cols tensor (shared by all cores) and per-core fp8 lhsT tensors."""
    kt = plan.kt
    fnT8 = (np.ascontiguousarray(fn_sorted.T) * np.float32(FSCALE)).astype(F8)

    cols = np.zeros((kt, KP, plan.col_slots), dtype=F8)
    for c in range(plan.ncls):
        nrows = plan.counts[c]
        src = fnT8[:, plan.cls_row_off[c]: plan.cls_row_off[c] + nrows]
        cols[:, :, plan.panel_off[c]: plan.panel_off[c] + nrows] = (
            src.reshape(kt, KP, nrows))

    lhsTs = []
    for core in range(NCORES):
        lt = np.zeros((kt, KP, plan.row_slots), dtype=F8)
        for c in range(plan.ncls):
            for j in range(plan.R[c]):
                t = plan.phys_tile(core, j)
                if t >= plan.RT[c]:
                    continue
                nreal = plan.realrows(c, t)
                src = fnT8[:, plan.cls_row_off[c] + KP * t:
                           plan.cls_row_off[c] + KP * t + nreal]
                off = plan.slot_off[c] + j * KP
                lt[:, :, off: off + nreal] = src.reshape(kt, KP, nreal)
        lhsTs.append(lt)
    return cols, lhsTs


def _dev_scale():
    """f32 scale applied inside the device exp: 1/(T * FSCALE^2)."""
    s = np.float32(1.0) / np.float32(TEMPERATURE)
    return np.float32(s / np.float32(FSCALE * FSCALE))


# --------------------------------------------------------------------------
# bass program
# --------------------------------------------------------------------------

def _build_program(plan, reps=1):
    import os
    probe = set(p for p in os.environ.get("KPROBE", "").split(",") if p)
    if "noact" in probe or "mmonly" in probe:
        probe |= {"nocsum"}
    if "mmonly" in probe:
        probe |= {"noact", "nodiag"}
    import concourse.bacc as bacc
    import concourse.tile as tile
    import concourse.mybir as mybir

    f32 = mybir.dt.float32
    f32r = mybir.dt.float32r
    f8 = mybir.dt.float8e4
    scale_dev = float(_dev_scale())
    kt = plan.kt
    kt2 = kt // 2

    nc = bacc.Bacc("TRN2", target_bir_lowering=False, debug=False)
    lhsT_d = nc.dram_tensor("lhsT", [kt, KP, plan.row_slots], f8,
                            kind="ExternalInput")
    cols_d = nc.dram_tensor("cols", [kt, KP, plan.col_slots], f8,
                            kind="ExternalInput")
    dmask_d = nc.dram_tensor("dmask", [KP, PANEL], f32, kind="ExternalInput")
    ones_d = nc.dram_tensor("ones", [KP, 1], f32r, kind="ExternalInput")
    part_d = nc.dram_tensor("partials", [KP, plan.npart], f32,
                            kind="ExternalOutput")
    diag_d = nc.dram_tensor("diag", [KP, plan.nrt, 2], f32,
                            kind="ExternalOutput")
    csum_d = nc.dram_tensor("csum", [plan.ncsum_pad, PANEL], f32,
                            kind="ExternalOutput")
    e0_d = nc.dram_tensor("e0", [KP, 1], f32, kind="ExternalOutput")
    SW = plan.setmax * PANEL

    DR = mybir.MatmulPerfMode.DoubleRow

    with tile.TileContext(nc) as tc:
        with (
            tc.tile_pool(name="persist", bufs=1) as persist,
            tc.tile_pool(name="work", bufs=4) as work,
            tc.tile_pool(name="psum", bufs=2, space="PSUM") as psum_main,
            tc.tile_pool(name="psumc", bufs=2, space="PSUM") as psum_cs,
        ):
            # whole fn matrix resident in SBUF (fp8): row-slot view (per-core
            # comb layout) + column-panel view (shared layout)
            lhsT = persist.tile([KP, kt, plan.row_slots], f8)
            for k in range(kt):
                nc.sync.dma_start(out=lhsT[:, k, :], in_=lhsT_d[k])
            cols_sb = persist.tile([KP, kt, plan.col_slots], f8)
            dmask = persist.tile([KP, PANEL], f32)
            nc.sync.dma_start(out=dmask, in_=dmask_d[:])
            ones = persist.tile([KP, 1], f32r)
            nc.sync.dma_start(out=ones, in_=ones_d[:])
            partials = persist.tile([KP, plan.npart], f32)
            diag = persist.tile([KP, plan.nrt, 2], f32)
            if "noact" in probe:
                nc.vector.memset(partials, 0.0)
            if "nodiag" in probe:
                nc.vector.memset(diag, 0.0)

            # exp(0) witness (dead-row correction on host)
            zt = persist.tile([KP, 1], f32)
            nc.vector.memset(zt, 0.0)
            e0t = persist.tile([KP, 1], f32)
            nc.scalar.activation(out=e0t, in_=zt,
                                 func=mybir.ActivationFunctionType.Exp,
                                 scale=scale_dev)
            nc.sync.dma_start(out=e0_d[:], in_=e0t)

            def emit_panel_dmas():
                for c, col0, colw, _sids in plan.emit_groups:
                    for k in range(kt):
                        nc.sync.dma_start(
                            out=cols_sb[:, k, col0:col0 + colw],
                            in_=cols_d[k, :, col0:col0 + colw])

            if "nopanels" in probe:
                emit_panel_dmas()

            def emit_body():
                # Pending colsums of set s are emitted after set s+1's main
                # matmuls (their exp runs on ScalarE during those matmuls, so
                # the in-order PE doesn't stall).  Colsum outputs are packed 4
                # per PSUM bank at base partitions 0/32/64/96 (distinct PE
                # column-groups, so the matmuls overlap in the array), then
                # DMA'd straight PSUM->DRAM.
                pending = []          # (e_tile, eoff, w, slot)
                CSB = 8
                batch = {"tile": None, "base": None, "n": 0}

                def flush_batch():
                    if batch["n"] and "nocsdma" not in probe:
                        b0 = batch["base"]
                        nc.sync.dma_start(
                            out=csum_d[b0:b0 + batch["n"], :],
                            in_=batch["tile"][0:1, :batch["n"] * PANEL])
                    batch["tile"] = None
                    batch["n"] = 0

                def flush_pending():
                    for e_, eoff_, w_, slot_ in pending:
                        pcs = psum_cs.tile([1, PANEL], f32, name="pcs")
                        nc.tensor.matmul(pcs[:, :w_], ones,
                                         e_[:, eoff_:eoff_ + w_],
                                         start=True, stop=True)
                        if batch["tile"] is None:
                            batch["tile"] = work.tile([1, CSB * PANEL], f32,
                                                      tag="csb", name="csb",
                                                      bufs=2)
                            batch["base"] = slot_
                        idx = slot_ - batch["base"]
                        nc.vector.tensor_copy(
                            batch["tile"][0:1, idx * PANEL:(idx + 1) * PANEL],
                            pcs[:, :])
                        batch["n"] = idx + 1
                        if batch["n"] == CSB:
                            flush_batch()
                    pending.clear()

                for c, col0, colw, sids in plan.emit_groups:
                    if "nopanels" not in probe:
                        for k in range(kt):
                            nc.sync.dma_start(
                                out=cols_sb[:, k, col0:col0 + colw],
                                in_=cols_d[k, :, col0:col0 + colw])
                    for si in sids:
                        c_, j, slices, pa, pb, span = plan.set_seq[si]
                        off = plan.slot_off[c] + j * KP
                        rsl = slice(off, off + KP)
                        ps = psum_main.tile([KP, SW], f32, name="ps")
                        # k2-major: runs of len(slices) same-weight matmuls
                        for k2 in range(kt2):
                            for p, so, w in slices:
                                c0 = plan.panel_off[c] + p * PANEL
                                nc.tensor.matmul(
                                    ps[:, so:so + w],
                                    lhsT[:, 2 * k2:2 * k2 + 2, rsl],
                                    cols_sb[:, 2 * k2:2 * k2 + 2, c0:c0 + w],
                                    start=(k2 == 0),
                                    stop=(k2 == kt2 - 1),
                                    perf_mode=DR)
                        # colsums of the previous set; their exp ran on
                        # ScalarE during our matmuls
                        if "nocsum" not in probe:
                            flush_pending()
                        e = work.tile([KP, SW], f32r, tag="etile", name="e")
                        if "noact" not in probe:
                            if pa is not None:
                                w0 = slices[0][2]
                                nc.scalar.activation(
                                    out=e[:, :w0], in_=ps[:, :w0],
                                    func=mybir.ActivationFunctionType.Exp,
                                    scale=scale_dev,
                                    accum_out=partials[:, pa:pa + 1])
                            if pb is not None:
                                b0 = slices[1][1] if pa is not None else 0
                                nc.scalar.activation(
                                    out=e[:, b0:span], in_=ps[:, b0:span],
                                    func=mybir.ActivationFunctionType.Exp,
                                    scale=scale_dev,
                                    accum_out=partials[:, pb:pb + 1])
                        if "nodiag" not in probe:
                            for p, so, w in slices:
                                if p not in (2 * j, 2 * j + 1):
                                    continue
                                parity = p - 2 * j
                                tmp = work.tile([KP, PANEL], f32, tag="dtmp",
                                                name="dtmp")
                                nc.vector.tensor_mul(tmp[:, :w],
                                                     ps[:, so:so + w],
                                                     dmask[:, :w])
                                t_idx = plan.rowtile_index(c, j)
                                nc.vector.reduce_sum(
                                    diag[:, t_idx, parity:parity + 1],
                                    tmp[:, :w],
                                    axis=mybir.AxisListType.X)
                        if "nocsum" not in probe:
                            for p, so, w in slices:
                                if p == 2 * j:
                                    continue
                                pending.append(
                                    (e, so, w, plan.csum_slot[(c_, p, j)]))
                flush_pending()
                flush_batch()

            if reps > 1:
                with tc.For_i(0, reps, 1):
                    emit_body()
            else:
                emit_body()

            nc.sync.dma_start(out=part_d[:], in_=partials)
            nc.sync.dma_start(out=diag_d[:], in_=diag)
    nc.compile()
    return nc


# --------------------------------------------------------------------------
# numpy simulation of the device outputs (for logic validation)
# --------------------------------------------------------------------------

def _simulate_device(plan, cols, lhsTs):
    scale_dev = _dev_scale()
    results = []
    kt = plan.kt
    colsf = cols.astype(np.float32).reshape(kt * KP, plan.col_slots)
    for core in range(NCORES):
        lt = lhsTs[core].astype(np.float32).reshape(kt * KP, plan.row_slots)
        partials = np.zeros((KP, plan.npart), dtype=np.float32)
        diag = np.zeros((KP, plan.nrt, 2), dtype=np.float32)
        csum = np.zeros((plan.ncsum_pad, PANEL), dtype=np.float32)
        doff = (core % 4) * KP
        for c, j, slices, pa, pb, span in plan.set_seq:
            off = plan.slot_off[c] + j * KP
            pbsum = np.zeros(KP, dtype=np.float32)
            for si_, (p, so, w) in enumerate(slices):
                c0 = plan.panel_off[c] + p * PANEL
                s = (lt[:, off:off + KP].T @ colsf[:, c0:c0 + w]
                     ).astype(np.float32)
                e = np.exp((s * scale_dev).astype(np.float32),
                           dtype=np.float32)
                rs = e.sum(axis=1, dtype=np.float32)
                if pa is not None and si_ == 0:
                    partials[:, pa] = rs
                else:
                    pbsum += rs
                if p != 2 * j:
                    csum[plan.csum_slot[(c, p, j)], :w] = e.sum(
                        axis=0, dtype=np.float32)
                if p in (2 * j, 2 * j + 1):
                    d = np.zeros(KP, dtype=np.float32)
                    n = max(0, min(KP, w - doff))
                    d[:n] = s[np.arange(n), doff + np.arange(n)]
                    diag[:, plan.rowtile_index(c, j), p - 2 * j] = d
            if pb is not None:
                partials[:, pb] = pbsum
        results.append({"partials": partials, "diag": diag, "csum": csum,
                        "e0": np.ones((KP, 1), dtype=np.float32)})
    return results


# --------------------------------------------------------------------------
# host-side finish
# --------------------------------------------------------------------------

def _finish(plan, results):
    """Combine per-core device outputs into the scalar loss (float64).

    Row i (class c, class-row g = 128*t + i, owner core k = t%8, j = t//8):
      denom_g = sum over sets of (c,j): pb (+ pa if core<4)                (rows)
              + sum over tiles t' with t'//4 < p_g of
                    csum[slot(c, p_g, j')] - dead(t')*e0                   (cols)
      x_g     = f32(diag[i, rowtile(c, t//8)] * f32(scale_dev))
      loss_g  = log(denom_g) - x_g
    """
    scale_dev = _dev_scale()
    sets_of = {}
    for si, (c, j, slices, pa, pb, span) in enumerate(plan.set_seq):
        sets_of.setdefault((c, j), []).append(si)
    total = 0.0
    nrows = 0
    for c in range(plan.ncls):
        cnt = plan.counts[c]
        denom = np.zeros(cnt, dtype=np.float64)
        x = np.zeros(cnt, dtype=np.float64)
        for core in range(NCORES):
            partials = results[core]["partials"].astype(np.float64)
            diag = results[core]["diag"]
            csum = results[core]["csum"].astype(np.float64)
            e0 = float(results[core]["e0"][0, 0])
            for j in range(plan.R[c]):
                t = plan.phys_tile(core, j)
                if t >= plan.RT[c]:
                    continue
                m = plan.realrows(c, t)
                rows = slice(KP * t, KP * t + m)
                # row-sum contributions.  Every set of (c,j) covers panels
                # p >= 2j; cores 0-3 need p >= 2j (pa+pb), cores 4-7 need
                # p >= 2j+1 (pb only).  The width-padded last panel is
                # covered exactly once -> subtract its fake column.
                for si in sets_of[(c, j)]:
                    _c, _j, _sl, pa, pb, _sp = plan.set_seq[si]
                    if pb is not None:
                        denom[rows] += partials[:m, pb]
                    if pa is not None and core < 4:
                        denom[rows] += partials[:m, pa]
                denom[rows] -= (plan.W[c] - plan.Wreal[c]) * e0
                # col-sum contributions: strictly-upper panels (p > t//4)
                for p in range(t // 4 + 1, plan.P[c]):
                    slot = plan.csum_slot.get((c, p, j))
                    if slot is None:
                        continue
                    wr = min(plan.width(c, p), plan.counts[c] - PANEL * p)
                    cols_sl = slice(PANEL * p, PANEL * p + wr)
                    dead = KP - m
                    denom[cols_sl] += csum[slot, :wr] - dead * e0
                # raw diagonal (parity: which of the two candidate panels
                # held this core's diagonal block)
                x[rows] = (diag[:m, plan.rowtile_index(c, j), core // 4]
                           .astype(np.float32) * scale_dev
                           ).astype(np.float32).astype(np.float64)
        total += float(np.sum(np.log(denom) - x))
        nrows += cnt
    assert nrows == plan.n2, (nrows, plan.n2)
    return np.float32(total / nrows)


# --------------------------------------------------------------------------
# entry point
# --------------------------------------------------------------------------

def _get_compiled(plan, reps=1):
    key = (plan.n2, plan.d, tuple(plan.counts), reps)
    if key not in _CACHE:
        _CACHE[key] = _build_program(plan, reps=reps)
    return _CACHE[key]


def _prepare(inputs):
    features = np.asarray(inputs["features"])
    labels = np.asarray(inputs["labels"])
    aug_indices = np.asarray(inputs["aug_indices"])

    fn, labs, perm, fn_sorted, labs_sorted = _host_prep(
        features, labels, aug_indices)
    n2, d = fn_sorted.shape
    classes, counts = np.unique(labs_sorted, return_counts=True)
    plan = _Plan(n2, d, counts.tolist())
    cols, lhsTs = _build_host_arrays(plan, fn_sorted)
    ones = np.ones((KP, 1), dtype=np.float32)
    in_maps = []
    for core in range(NCORES):
        dmask = np.zeros((KP, PANEL), dtype=np.float32)
        off = (core % 4) * KP
        dmask[np.arange(KP), off + np.arange(KP)] = 1.0
        in_maps.append({"lhsT": lhsTs[core], "cols": cols, "dmask": dmask,
                        "ones": ones})
    return plan, cols, lhsTs, in_maps


def kernel(simulate=False, **inputs):
    plan, cols, lhsTs, in_maps = _prepare(inputs)

    if simulate:
        results = _simulate_device(plan, cols, lhsTs)
    else:
        from concourse.bass_utils import run_bass_kernel_spmd

        nc = _get_compiled(plan)
        results = run_bass_kernel_spmd(nc, in_maps,
                                       core_ids=list(range(NCORES))).results

    return np.asarray(_finish(plan, results), dtype=np.float32)


# --------------------------------------------------------------------------
# timing harness (mirrors bass2jax.run_bass_via_pjrt's multi-core path but
# keeps the big inputs device-resident so repeated calls time the NEFF)
# --------------------------------------------------------------------------

def _make_sharded(nc, n_cores):
    import jax
    import concourse.mybir as mybir
    from jax.sharding import Mesh, PartitionSpec
    from jax.experimental.shard_map import shard_map
    from concourse.bass2jax import (_bass_exec_p, install_neuronx_cc_hook,
                                    partition_id_tensor)

    install_neuronx_cc_hook()
    partition_name = (nc.partition_id_tensor.name
                      if nc.partition_id_tensor else None)
    in_names, out_names, out_avals, zero_outs = [], [], [], []
    for alloc in nc.m.functions[0].allocations:
        if not isinstance(alloc, mybir.MemoryLocationSet):
            continue
        name = alloc.memorylocations[0].name
        if alloc.kind == "ExternalInput":
            if name != partition_name:
                in_names.append(name)
        elif alloc.kind == "ExternalOutput":
            out_names.append(name)
            shape = tuple(alloc.tensor_shape)
            dtype = mybir.dt.np(alloc.dtype)
            out_avals.append(jax.core.ShapedArray(shape, dtype))
            zero_outs.append(np.zeros(shape, dtype))
    n_params = len(in_names)
    all_names = in_names + out_names
    if partition_name is not None:
        all_names.append(partition_name)

    def _body(*args):
        operands = list(args)
        if partition_name is not None:
            operands.append(partition_id_tensor())
        outs = _bass_exec_p.bind(
            *operands,
            out_avals=tuple(out_avals),
            in_names=tuple(all_names),
            out_names=tuple(out_names),
            lowering_input_output_aliases=(),
            sim_require_finite=True,
            sim_require_nnan=True,
            nc=nc,
        )
        return tuple(outs)

    devices = jax.devices()[:n_cores]
    mesh = Mesh(np.asarray(devices), ("core",))
    in_specs = (PartitionSpec("core"),) * (n_params + len(out_names))
    out_specs = (PartitionSpec("core"),) * len(out_names)
    donate = tuple(range(n_params, n_params + len(out_names)))
    sharded = jax.jit(
        shard_map(_body, mesh=mesh, in_specs=in_specs, out_specs=out_specs,
                  check_rep=False),
        donate_argnums=donate, keep_unused=True)
    return sharded, in_names, out_names, out_avals, zero_outs, mesh


def _make_runner(nc, in_maps):
    import jax
    from jax.sharding import NamedSharding, PartitionSpec

    sharded, in_names, out_names, out_avals, zero_outs, mesh = _make_sharded(
        nc, NCORES)
    concat_in = [np.concatenate([in_maps[c][n] for c in range(NCORES)], axis=0)
                 for n in in_names]
    sharding = NamedSharding(mesh, PartitionSpec("core"))
    dev_in = [jax.device_put(a, sharding) for a in concat_in]

    def run():
        import time
        zs = [jax.device_put(
            np.zeros((NCORES * z.shape[0], *z.shape[1:]), z.dtype), sharding)
            for z in zero_outs]
        jax.block_until_ready(zs)
        t0 = time.perf_counter()
        out = sharded(*dev_in, *zs)
        jax.block_until_ready(out)
        return time.perf_counter() - t0

    run()  # warmup (compile + first exec)
    return run


def benchmark(loop_reps=129, pairs=10, **inputs):
    """Per-iteration kernel time, cancelling the ~100ms axon dispatch floor:
    interleave timings of a 1-rep NEFF and a `loop_reps`-rep NEFF (HW loop)
    and difference the minima."""
    plan, cols, lhsTs, in_maps = _prepare(inputs)
    run1 = _make_runner(_get_compiled(plan, reps=1), in_maps)
    runR = _make_runner(_get_compiled(plan, reps=loop_reps), in_maps)

    t1s, tRs = [], []
    for _ in range(pairs):
        t1s.append(run1())
        tRs.append(runR())
    m1, mR = min(t1s), min(tRs)
    per_iter = (mR - m1) / (loop_reps - 1)
    print(f"  [bench] min T(1)={m1*1e3:.2f}ms  min T({loop_reps})={mR*1e3:.2f}ms")
    return per_iter * 1e9
